# revision 1
# baseline (speedup 1.0000x reference)
"""Expert-choice MoE kernel for 8 Trainium2 NeuronCores (expert-parallel).

Decomposition (core e handles expert e):
  - router logits x . emb_e computed in fp32 on PE; top-8 token indices per
    batch row via DVE max8/max_index; token gather via indirect DMA.
  - sum_weights GEMM1 column-sharded (each core owns 1536 columns of sw_w1);
    the tiny (8,64) partial logits are AllReduced, softmaxed locally.
  - expert MLP: GEMM1 (w1) in bf16; GEMM2 (w2) weights streamed as fp8e3m4
    scaled x128 (descale folded into the combine weights `wes`).
  - er * w[:, e] contributions AllReduced in bf16 in one shot (chunked
    ARs would just serialize on the CC engine behind the z-AR).
  - classification head sharded: GEMM1 column-shard (384 cols of ch_w1),
    GEMM2 contraction-shard (384 rows of ch_w2); per-core (64,1000) partials
    are summed on the host (+ ch_b2).

DMA ring assignment: sync(HWDGE/SP) ring carries ONLY the big weight
stream (sw1 -> w1 -> w2) in consumption order, packed into large
contiguous chunks (1.5MB sw / 1.5MB w1 / 1MB w2). Activations, inputs,
ch1/ch2 and collective staging ride the scalar(Act) ring; gathers and
collective triggers ride gpsimd (SWDGE).

PSUM plan (8 banks): tag "pm" bufs=6 + tag "pt" bufs=2 (transposes +
router). The "pm" ring allocation order is load-bearing (ring reuse must
only ever land on a dead tile): pms x3, pz, pme x6, pme2 x6, pmh, pmo x2.
"""

import numpy as np
import ml_dtypes

import concourse.bass as bass
from concourse import bacc
import concourse.mybir as mybir
import concourse.tile as tile
from concourse.bass import ts, ds
from concourse.bass_utils import run_bass_kernel_spmd
from concourse.masks import make_identity

B, N, D, E, K, C = 64, 32, 384, 8, 8, 1000
KD, ND = K * D, N * D          # 3072, 12288
P = 128
NTOK = B * N                   # 2048
SWC = ND // E                  # 1536 sum-weights columns per core
CH1C = KD // E                 # 384 head-GEMM1 columns per core
KCE = KD // P                  # 24 k-chunks, expert GEMMs
KCS = ND // P                  # 96 k-chunks, sum-weights GEMM1
KCH = SWC // P                 # 12 k-chunks, z GEMM
NCORES = 8

SWPACK = 4                     # k-chunks per sw1 DMA (1.5MB)
NSW = KCS // SWPACK            # 24 sw tiles
W1PACK = 2                     # k-chunks per w1 DMA (1.5MB)
NW1 = KCE // W1PACK            # 12 w1 tiles
W2SUB = 6                      # w2 sub-DMAs (full-width, 4 k-chunks each)
W2K = KCE // W2SUB             # 4 k-chunks per w2 sub-DMA

W2_FP8 = True                  # stream w2 as fp8e3m4 (scale 128)
W2_SCALE = 128.0
NW1_BF = 6                     # w1 tiles in bf16; the rest stream as fp8e3
W1_E3_KC0 = NW1_BF * W1PACK    # first k-chunk whose w1 weights are fp8e3

F32 = mybir.dt.float32
BF16 = mybir.dt.bfloat16
FP8E3 = mybir.dt.float8e3
U32 = mybir.dt.uint32
GELU = mybir.ActivationFunctionType.Gelu
EXP = mybir.ActivationFunctionType.Exp
X_AX = mybir.AxisListType.X
ADD = mybir.AluOpType.add
bf16 = ml_dtypes.bfloat16
f8e3 = ml_dtypes.float8_e3m4

W2DT = FP8E3 if W2_FP8 else BF16


def _build(include_bias: bool) -> bass.Bass:
    nc = bacc.Bacc("TRN2", num_devices=NCORES)

    # weight stream (sync ring), packed layouts produced by _pack_inputs
    swd = nc.dram_tensor("swd", [NSW * P, SWPACK * SWC], BF16, kind="ExternalInput")
    w1d = nc.dram_tensor("w1d", [NW1_BF * P, W1PACK * KD], BF16, kind="ExternalInput")
    w1f = nc.dram_tensor("w1f", [(NW1 - NW1_BF) * P, W1PACK * KD], FP8E3,
                         kind="ExternalInput")
    w2d = nc.dram_tensor("w2d", [W2SUB * P, W2K * KD], W2DT, kind="ExternalInput")
    # everything else (scalar/gpsimd rings)
    xt = nc.dram_tensor("xt", [P, 3, NTOK + 1], F32, kind="ExternalInput")
    x2b = nc.dram_tensor("x2b", [NTOK, D], BF16, kind="ExternalInput")
    xft = nc.dram_tensor("xft", [P, KCS, B], BF16, kind="ExternalInput")
    sw2 = nc.dram_tensor("sw2", [P, KCH, E], BF16, kind="ExternalInput")
    ch1 = nc.dram_tensor("ch1", [P, KCE * CH1C], BF16, kind="ExternalInput")
    ch2 = nc.dram_tensor("ch2", [P, 3, C], BF16, kind="ExternalInput")
    oh = nc.dram_tensor("oh", [B, E], F32, kind="ExternalInput")
    if include_bias:
        b1d = nc.dram_tensor("b1d", [1, KD], F32, kind="ExternalInput")
        b2d = nc.dram_tensor("b2d", [1, KD], F32, kind="ExternalInput")  # pre-scaled
        swb1d = nc.dram_tensor("swb1d", [1, SWC], F32, kind="ExternalInput")
        swb2d = nc.dram_tensor("swb2d", [1, E], F32, kind="ExternalInput")
        chb1d = nc.dram_tensor("chb1d", [1, CH1C], F32, kind="ExternalInput")
    outp = nc.dram_tensor("outp", [B, C], F32, kind="ExternalOutput")

    with tile.TileContext(nc) as tc:
        with (
            tc.tile_pool(name="consts", bufs=1) as consts,
            tc.tile_pool(name="acts", bufs=1) as acts,
            tc.tile_pool(name="wpool", bufs=7) as wpool,
            tc.tile_pool(name="ps", bufs=6, space="PSUM") as ps,
            tc.tile_pool(name="dram", bufs=1, space="DRAM") as dram,
        ):
            # ---- constants / inputs on the scalar+gpsimd rings ----
            ident = consts.tile([P, P], BF16)
            make_identity(nc, ident[:])
            xft_sb = consts.tile([P, KCS, B], BF16)
            nc.scalar.dma_start(xft_sb[:], xft[:])
            xt_sb = acts.tile([P, 3, NTOK + 1], F32)
            nc.scalar.dma_start(xt_sb[:], xt[:])
            sw2_sb = consts.tile([P, KCH, E], BF16)
            nc.scalar.dma_start(sw2_sb[:], sw2[:])
            oh_sb = consts.tile([B, E], F32)
            nc.scalar.dma_start(oh_sb[:], oh[:])
            ch2_sb = consts.tile([P, 3, C], BF16)
            nc.scalar.dma_start(ch2_sb[:], ch2[:])
            if include_bias:
                b1_sb = consts.tile([B, KD], F32)
                nc.scalar.dma_start(b1_sb[:], b1d[0:1, :].to_broadcast([B, KD]))
                b2_sb = consts.tile([B, KD], F32)
                nc.scalar.dma_start(b2_sb[:], b2d[0:1, :].to_broadcast([B, KD]))
                swb1_sb = consts.tile([B, SWC], F32)
                nc.scalar.dma_start(swb1_sb[:], swb1d[0:1, :].to_broadcast([B, SWC]))
                swb2_sb = consts.tile([B, E], F32)
                nc.scalar.dma_start(swb2_sb[:], swb2d[0:1, :].to_broadcast([B, E]))
                chb1_sb = consts.tile([B, CH1C], F32)
                nc.scalar.dma_start(chb1_sb[:], chb1d[0:1, :].to_broadcast([B, CH1C]))

            pwarm = ps.tile([P, B], BF16, name="pwarm", tag="pt", bufs=2)
            nc.tensor.transpose(pwarm[:32, :32], ident[:32, :32], ident[:32, :32])

            # ---- sum-weights GEMM1: stream sw1, accumulate 3 banks ----
            pms = [ps.tile([B, 512], F32, name=f"pms{n}", tag="pm")
                   for n in range(3)]
            for t in range(NSW):
                wt = wpool.tile([P, SWPACK * SWC], BF16, name="wt", tag="wt")
                nc.sync.dma_start(wt[:], swd[ts(t, P), :])
                for k in range(SWPACK):
                    c = t * SWPACK + k
                    for n in range(3):
                        nc.tensor.matmul(
                            pms[n][:], xft_sb[:, c, :],
                            wt[:, ds(k * SWC + n * 512, 512)],
                            start=(c == 0), stop=(c == KCS - 1),
                        )
                if t == 1:
                    # ---- router (after xt lands): logits = x @ emb_e ----
                    lg_flat = acts.tile([1, NTOK], F32)
                    for nt in range(4):
                        pr = ps.tile([B, 512], F32, name="pr", tag="pt", bufs=2)
                        for cc in range(3):
                            nc.tensor.matmul(
                                pr[:1, :], xt_sb[:, cc, NTOK : NTOK + 1],
                                xt_sb[:, cc, ts(nt, 512)],
                                start=(cc == 0), stop=(cc == 2),
                            )
                        nc.vector.tensor_copy(lg_flat[:, ts(nt, 512)], pr[:1, :])
                    lg_dram = dram.tile([1, NTOK], F32)
                    nc.scalar.dma_start(lg_dram[:], lg_flat[:])
                    lg_bn = acts.tile([B, N], F32)
                    nc.scalar.dma_start(
                        lg_bn[:], lg_dram[:].rearrange("x (b n) -> (x b) n", b=B))
                if t == 2:
                    # ---- top-8 per row + token gather ----
                    vals8 = acts.tile([B, 8], F32)
                    idx8 = acts.tile([B, 8], U32)
                    nc.vector.max(out=vals8[:], in_=lg_bn[:])
                    nc.vector.max_index(out=idx8[:], in_max=vals8[:], in_values=lg_bn[:])
                    base = acts.tile([B, 1], U32)
                    nc.gpsimd.iota(base[:], pattern=[[0, 1]], base=0, channel_multiplier=N)
                    off = acts.tile([B, 8], U32)
                    nc.vector.tensor_tensor(
                        out=off[:], in0=idx8[:], in1=base[:].to_broadcast([B, 8]), op=ADD)
                    sel = acts.tile([B, K, D], BF16)
                    for k in range(K):
                        nc.gpsimd.indirect_dma_start(
                            out=sel[:, k, :], out_offset=None,
                            in_=x2b[:],
                            in_offset=bass.IndirectOffsetOnAxis(ap=off[:, k : k + 1], axis=0),
                        )
                if t == 3:
                    # selT chunks [128, 24, 64] for expert GEMM1 stationaries.
                    # Chunks >= W1_E3_KC0 are pre-scaled by 1/W2_SCALE so the
                    # fp8e3 (x128) w1 tiles accumulate at the same PSUM scale
                    # as the bf16 ones.
                    sel_flat = sel[:].rearrange("b k d -> b (k d)")
                    selT = acts.tile([P, KCE, B], BF16)
                    for c in range(KCE):
                        pt = ps.tile([P, B], BF16, name="pt", tag="pt", bufs=2)
                        nc.tensor.transpose(pt[:], sel_flat[:, ts(c, P)], ident[:B, :B])
                        if c >= W1_E3_KC0:
                            nc.vector.tensor_scalar_mul(
                                selT[:, c, :], pt[:], 1.0 / W2_SCALE)
                        else:
                            nc.vector.tensor_copy(selT[:, c, :], pt[:])

            # ---- z = h1 @ sw2 partials, AllReduce (tiny, fp32) ----
            h1 = acts.tile([B, SWC], BF16)
            for n in range(3):
                if include_bias:
                    nc.vector.tensor_add(pms[n][:], pms[n][:], swb1_sb[:, ts(n, 512)])
                nc.scalar.activation(h1[:, ts(n, 512)], pms[n][:], GELU)
            h1T = acts.tile([P, KCH, B], BF16)
            for c in range(KCH):
                pt = ps.tile([P, B], BF16, name="pt", tag="pt", bufs=2)
                nc.tensor.transpose(pt[:], h1[:, ts(c, P)], ident[:B, :B])
                nc.vector.tensor_copy(h1T[:, c, :], pt[:])
            pz = ps.tile([B, 512], F32, name="pz", tag="pm")
            for c in range(KCH):
                nc.tensor.matmul(
                    pz[:E, :B], sw2_sb[:, c, :], h1T[:, c, :],
                    start=(c == 0), stop=(c == KCH - 1),
                )
            zT_sb = acts.tile([E, B], F32)
            nc.vector.tensor_copy(zT_sb[:], pz[:E, :B])
            zin = dram.tile([E, B], F32)
            zout = dram.tile([E, B], F32)
            nc.scalar.dma_start(zin[:], zT_sb[:])
            nc.gpsimd.collective_compute(
                "AllReduce", ADD, replica_groups=[list(range(NCORES))],
                ins=[zin[:].opt()], outs=[zout[:].opt()],
            )

            # ---- expert GEMM1: h = gelu(selT.T @ w1_e) ----
            h = acts.tile([B, KD], BF16)
            pme = [ps.tile([B, 512], F32, name=f"pme{n}", tag="pm")
                   for n in range(6)]
            for t in range(NW1):
                if t < NW1_BF:
                    wt = wpool.tile([P, W1PACK * KD], BF16, name="wt", tag="wt")
                    nc.sync.dma_start(wt[:], w1d[ts(t, P), :])
                else:
                    wt = wpool.tile([P, W1PACK * KD], FP8E3, name="wt", tag="wt")
                    nc.sync.dma_start(wt[:], w1f[ts(t - NW1_BF, P), :])
                if t == 1:
                    # prefetch ch1 on the scalar ring (needed from the head on)
                    ch1_sb = consts.tile([P, KCE * CH1C], BF16)
                    nc.scalar.dma_start(ch1_sb[:], ch1[:])
                for k in range(W1PACK):
                    c = t * W1PACK + k
                    for n in range(6):
                        nc.tensor.matmul(
                            pme[n][:], selT[:, c, :],
                            wt[:, ds(k * KD + n * 512, 512)],
                            start=(c == 0), stop=(c == KCE - 1),
                        )
            last_gelu = None
            for n in range(6):
                if include_bias:
                    nc.vector.tensor_add(pme[n][:], pme[n][:], b1_sb[:, ts(n, 512)])
                last_gelu = nc.scalar.activation(h[:, ts(n, 512)], pme[n][:], GELU)
            hT = acts.tile([P, KCE, B], BF16)
            last_htc = None
            for c in range(KCE):
                pt = ps.tile([P, B], BF16, name="pt", tag="pt", bufs=2)
                nc.tensor.transpose(pt[:], h[:, ts(c, P)], ident[:B, :B])
                last_htc = nc.vector.tensor_copy(hT[:, c, :], pt[:])

            # ---- softmax over experts; wes = w[:, e] / W2_SCALE.  Emitted
            # after the h gelus/hT copies so no z-dependent op can park the
            # ACT or DVE FIFOs while GEMM1 output processing is pending. ----
            zb = acts.tile([B, E], F32)
            nc.gpsimd.dma_start(zb[:], zout[:].rearrange("e b -> b e"))
            if include_bias:
                nc.vector.tensor_add(zb[:], zb[:], swb2_sb[:])
            mx = acts.tile([B, 1], F32)
            mx_i = nc.vector.reduce_max(mx[:], zb[:], axis=X_AX)
            tile.add_dep_helper(mx_i.ins, last_htc.ins, sync=False,
                                reason="softmax after hT copies on DVE")
            nmx = acts.tile([B, 1], F32)
            nc.vector.tensor_scalar_mul(nmx[:], mx[:], -1.0)
            exps = acts.tile([B, E], F32)
            exp_i = nc.scalar.activation(exps[:], zb[:], EXP, bias=nmx[:])
            tile.add_dep_helper(exp_i.ins, last_gelu.ins, sync=False,
                                reason="Exp after expert gelus on ACT")
            sm = acts.tile([B, 1], F32)
            nc.vector.reduce_sum(sm[:], exps[:], axis=X_AX)
            rs = acts.tile([B, 1], F32)
            nc.vector.reciprocal(rs[:], sm[:])
            wv = acts.tile([B, E], F32)
            nc.vector.tensor_scalar_mul(wv[:], exps[:], rs[:])
            t8 = acts.tile([B, E], F32)
            nc.vector.tensor_mul(out=t8[:], in0=wv[:], in1=oh_sb[:])
            wes = acts.tile([B, 1], F32)
            nc.vector.reduce_sum(wes[:], t8[:], axis=X_AX)
            if W2_FP8:
                nc.vector.tensor_scalar_mul(wes[:], wes[:], 1.0 / W2_SCALE)

            # ---- expert GEMM2, single pass over all 3072 columns with 6
            # PSUM banks (same stationary amortization as GEMM1); ONE bf16
            # AllReduce for the whole [B, KD] er (chunked ARs would only
            # serialize on the CC engine behind the z-AR anyway, and each
            # extra AR pays its own mesh latency). ----
            cin = dram.tile([B, KD], BF16, name="cin")
            wsout = dram.tile([B, KD], BF16, name="wsout")
            er_bf = acts.tile([B, KD], BF16)
            pme2 = [ps.tile([B, 512], F32, name=f"pme2{n}", tag="pm")
                    for n in range(6)]
            for s in range(W2SUB):
                wt = wpool.tile([P, W2K * KD], W2DT, name="wt", tag="wt")
                nc.sync.dma_start(wt[:], w2d[ts(s, P), :])
                for k in range(W2K):
                    c = s * W2K + k
                    for n in range(6):
                        nc.tensor.matmul(
                            pme2[n][:], hT[:, c, :],
                            wt[:, ds(k * KD + n * 512, 512)],
                            start=(c == 0), stop=(c == KCE - 1),
                        )
            for n in range(6):
                if include_bias:
                    nc.vector.tensor_add(
                        pme2[n][:], pme2[n][:], b2_sb[:, ts(n, 512)])
                nc.vector.tensor_scalar_mul(
                    er_bf[:, ts(n, 512)], pme2[n][:], wes[:])
            nc.scalar.dma_start(cin[:], er_bf[:])
            pmh = ps.tile([B, 512], F32, name="pmh", tag="pm")
            nc.gpsimd.collective_compute(
                "AllReduce", ADD, replica_groups=[list(range(NCORES))],
                ins=[cin[:].opt()], outs=[wsout[:].opt()],
            )

            # ---- head GEMM1 (column shard): wsT loaded straight from the
            # AllReduce output in DRAM via one XBAR DMA transpose ----
            wsT = acts.tile([P, KCE, B], BF16)
            nc.scalar.dma_start(wsT[:], wsout[:], transpose=True)
            for c in range(KCE):
                nc.tensor.matmul(
                    pmh[:, :CH1C], wsT[:, c, :],
                    ch1_sb[:, ds(c * CH1C, CH1C)],
                    start=(c == 0), stop=(c == KCE - 1),
                )
            hh = acts.tile([B, CH1C], BF16)
            if include_bias:
                nc.vector.tensor_add(pmh[:, :CH1C], pmh[:, :CH1C], chb1_sb[:])
            nc.scalar.activation(hh[:], pmh[:, :CH1C], GELU)

            # ---- head GEMM2 (contraction shard): out_part = hh @ ch2_e ----
            hhT = acts.tile([P, 3, B], BF16)
            nc.scalar.dma_start(hhT[:], hh[:], transpose=True)
            outsb = acts.tile([B, C], F32)
            for nn in range(2):
                pmo = ps.tile([B, 512], F32, name="pmo", tag="pm")
                for c in range(3):
                    nc.tensor.matmul(
                        pmo[:, :500], hhT[:, c, :], ch2_sb[:, c, ds(nn * 500, 500)],
                        start=(c == 0), stop=(c == 2),
                    )
                nc.vector.tensor_copy(outsb[:, ds(nn * 500, 500)], pmo[:, :500])
            nc.scalar.dma_start(outp[:], outsb[:])

    nc.finalize()
    return nc


_NC_CACHE: dict = {}


def _get_nc(include_bias: bool) -> bass.Bass:
    if include_bias not in _NC_CACHE:
        _NC_CACHE[include_bias] = _build(include_bias)
    return _NC_CACHE[include_bias]


def _pack_w2(w2_e: np.ndarray) -> np.ndarray:
    """[3072, 3072] -> [W2SUB*128, W2K*3072], k-chunks packed per sub-DMA."""
    out = (w2_e.reshape(W2SUB, W2K, P, KD).transpose(0, 2, 1, 3)
           .reshape(W2SUB * P, W2K * KD))
    if W2_FP8:
        m = float(ml_dtypes.finfo(f8e3).max)
        return np.clip(out * W2_SCALE, -m, m).astype(f8e3)
    return np.ascontiguousarray(out).astype(bf16)


def _pack_inputs(inputs: dict, include_bias: bool) -> list[dict]:
    f32 = np.float32
    x = np.ascontiguousarray(np.asarray(inputs["x"], dtype=f32))      # (64,32,384)
    expert_emb = np.asarray(inputs["expert_emb"], dtype=f32)          # (8,384)
    w1 = np.asarray(inputs["w1"])                                     # (8,3072,3072)
    w2 = np.asarray(inputs["w2"])
    sw_w1 = np.asarray(inputs["sw_w1"])                               # (12288,12288)
    sw_w2 = np.asarray(inputs["sw_w2"])                               # (12288,8)
    ch_w1 = np.asarray(inputs["ch_w1"])                               # (3072,3072)
    ch_w2 = np.asarray(inputs["ch_w2"])                               # (3072,1000)

    x2 = x.reshape(NTOK, D)
    xt_base = x2.T.reshape(3, P, NTOK).transpose(1, 0, 2)             # (128,3,2048)
    x2b = x2.astype(bf16)                                             # (2048,384)
    xf = x.reshape(B, ND)
    xft_p = np.ascontiguousarray(
        xf.T.reshape(KCS, P, B).transpose(1, 0, 2)).astype(bf16)      # (128,96,64)

    ch1_full = ch_w1.reshape(KD, E, CH1C)                             # col shards
    ch2_full = ch_w2.reshape(E, CH1C, C)                              # row shards

    in_maps = []
    for e in range(NCORES):
        emb_p = expert_emb[e].reshape(3, P).T                          # (128,3)
        xt_p = np.ascontiguousarray(
            np.concatenate([xt_base, emb_p[:, :, None]], axis=2), dtype=f32)
        sw1_e = sw_w1[:, e * SWC:(e + 1) * SWC]                        # (12288,1536)
        swd_p = np.ascontiguousarray(
            sw1_e.reshape(NSW, SWPACK, P, SWC).transpose(0, 2, 1, 3)
            .reshape(NSW * P, SWPACK * SWC)).astype(bf16)
        w1_pack = np.ascontiguousarray(
            np.asarray(w1[e], f32).reshape(NW1, W1PACK, P, KD)
            .transpose(0, 2, 1, 3).reshape(NW1 * P, W1PACK * KD))
        w1d_p = w1_pack[:NW1_BF * P].astype(bf16)
        m8 = float(ml_dtypes.finfo(f8e3).max)
        w1f_p = np.clip(w1_pack[NW1_BF * P:] * W2_SCALE, -m8, m8).astype(f8e3)
        w2d_p = _pack_w2(np.asarray(w2[e], f32))
        sw2_e = np.ascontiguousarray(sw_w2[e * SWC:(e + 1) * SWC, :])  # (1536,8)
        sw2_p = np.ascontiguousarray(
            sw2_e.reshape(KCH, P, E).transpose(1, 0, 2)).astype(bf16)  # (128,12,8)
        ch1_p = np.ascontiguousarray(
            ch1_full[:, e, :].reshape(KCE, P, CH1C).transpose(1, 0, 2)
            .reshape(P, KCE * CH1C)).astype(bf16)                      # (128,24*384)
        ch2_p = np.ascontiguousarray(
            ch2_full[e].reshape(3, P, C).transpose(1, 0, 2)).astype(bf16)  # (128,3,1000)
        oh_p = np.zeros((B, E), dtype=f32)
        oh_p[:, e] = 1.0
        m = {
            "xt": xt_p, "x2b": x2b, "xft": xft_p,
            "swd": swd_p, "w1d": w1d_p, "w1f": w1f_p, "w2d": w2d_p,
            "sw2": sw2_p, "ch1": ch1_p, "ch2": ch2_p, "oh": oh_p,
        }
        if include_bias:
            m["b1d"] = np.asarray(inputs["b1"][e], f32).reshape(1, KD)
            b2v = np.asarray(inputs["b2"][e], f32).reshape(1, KD)
            m["b2d"] = b2v * (W2_SCALE if W2_FP8 else 1.0)
            m["swb1d"] = np.asarray(
                inputs["sw_b1"], f32).reshape(1, ND)[:, e * SWC:(e + 1) * SWC]
            m["swb2d"] = np.asarray(inputs["sw_b2"], f32).reshape(1, E)
            m["chb1d"] = np.asarray(
                inputs["ch_b1"], f32).reshape(1, KD)[:, e * CH1C:(e + 1) * CH1C]
        in_maps.append(m)
    return in_maps


def _need_bias(inputs) -> bool:
    return any(
        float(np.abs(np.asarray(inputs[k])).max()) != 0.0
        for k in ("b1", "b2", "sw_b1", "sw_b2", "ch_b1")
    )


def run(inputs: dict, **run_kwargs):
    """Run on the 8 cores; returns (full_output, BassKernelResults)."""
    include_bias = _need_bias(inputs)
    nc = _get_nc(include_bias)
    in_maps = _pack_inputs(inputs, include_bias)
    res = run_bass_kernel_spmd(nc, in_maps, core_ids=list(range(NCORES)), **run_kwargs)
    out = np.zeros((B, C), dtype=np.float64)
    for e in range(NCORES):
        out += res.results[e]["outp"].astype(np.float64)
    out += np.asarray(inputs["ch_b2"], np.float64)
    return out.astype(np.float32), res


def kernel(**inputs) -> np.ndarray:
    out, _ = run(inputs)
    return out



# revision 32
# speedup vs baseline: 1.1388x; 1.1388x over previous
"""Expert-choice MoE kernel for 8 Trainium2 NeuronCores (expert-parallel), v2.

Core e handles expert e. Changes vs v1 baseline (293us):
  - f16 replaces bf16 everywhere (8x finer mantissa, same bytes) -> the
    quantization floor drops 0.008 -> 0.001, freeing error budget for fp8.
  - w1 fully fp8e3m4 (x128 scale; x prescaled 1/128 on host), w2 fully fp8
    with a rank-1 mean-correction: the er error from fp8(w2) has a
    predictable component hbar[b]*colsum(delta2)[o]; colsum(delta2) is known
    at pack time and hbar comes from a ones-vector matmul over hT, so one
    extra K=1 contraction row per PSUM bank cancels it (emulated rel err
    0.0214 -> 0.0184).
  - sw_w1: first 48 k-chunks fp8, last 48 f16 (x128 scale, xft prescaled).
  - matmul column-pairing: all big GEMMs have M=64 (half the PE array).
    Two independent matmuls run concurrently via tile_position (0,0)/(0,64)
    writing psum partitions [0:64]/[64:128]. Only the first MM touching a
    bank uses start=True (the whole-bank has_written clear), the last uses
    stop=True; interleaved halves rely on per-element has_written semantics.
    Activations carry the half-split layout: SBUF [128, X] tiles hold
    column-block A on partitions 0:64 and block B on 64:128; PE transposes
    from the bottom half use ident[64:128, 64:128] and tile_position row 64.
  - er combine: ReduceScatter(+)->AllGather instead of AllReduce (measured
    ~16us vs ~33us at 384KB on 8 cores).
  - w2 streamed n-major so GEMM2 pairs (block r, block r+3) alternate.
  - router stays true fp32 (rank-order flips in top-8 are catastrophic).
"""

import os

import numpy as np
import ml_dtypes

import concourse.bass as bass
from concourse import bacc
import concourse.mybir as mybir
import concourse.tile as tile
from concourse.bass import ts, ds
from concourse.bass_utils import run_bass_kernel_spmd
from concourse.masks import make_identity

B, N, D, E, K, C = 64, 32, 384, 8, 8, 1000
KD, ND = K * D, N * D          # 3072, 12288
P = 128
NTOK = B * N                   # 2048
SWC = ND // E                  # 1536 sum-weights columns per core
CH1C = KD // E                 # 384 head-GEMM1 columns per core
KCE = KD // P                  # 24 k-chunks, expert GEMMs
KCS = ND // P                  # 96 k-chunks, sum-weights GEMM1
KCH = SWC // P                 # 12 k-chunks, z GEMM
NCORES = 8

SCALE = 128.0                  # fp8/f16 weight scale (activations pre-divided)
# sw1 columns split by importance (||sw_w2[col,:]||): the 768 least
# important columns stream fp8, the rest f16 -- a pack-time column
# permutation of sw1 (+ identical sw2 row permutation) makes both groups
# contiguous. Each group covers ALL 96 k-chunks.
SW8C = SWC // 2                # 768 fp8 columns
SW16C = SWC - SW8C             # 768 f16 columns
PACK8 = 16                     # k-chunks per fp8 sw tile (1.57MB)
PACK16 = 8                     # k-chunks per f16 sw tile (1.57MB)
NSW8 = KCS // PACK8            # 6
NSW16 = KCS // PACK16          # 12
W1PACK = 4                     # k-chunks per w1 fp8 tile (1.57MB)
NW1 = KCE // W1PACK            # 6
NW2 = 3                        # w2 tiles, n-major pairs (3.14MB each)

F32 = mybir.dt.float32
F16 = mybir.dt.float16
FP8E3 = mybir.dt.float8e3
U32 = mybir.dt.uint32
GELU = mybir.ActivationFunctionType.Gelu
EXP = mybir.ActivationFunctionType.Exp
X_AX = mybir.AxisListType.X
ADD = mybir.AluOpType.add
f16 = np.float16
f8e3 = ml_dtypes.float8_e3m4
M8 = float(ml_dtypes.finfo(f8e3).max)


DEBUG_DUMP = os.environ.get("KDEBUG", "0") == "1"


def _build(include_bias: bool) -> bass.Bass:
    nc = bacc.Bacc("TRN2", num_devices=NCORES)

    # weight stream (sync ring), packed layouts from _pack_inputs
    swd8 = nc.dram_tensor("swd8", [NSW8 * P, PACK8 * SW8C], FP8E3, kind="ExternalInput")
    swd16 = nc.dram_tensor("swd16", [NSW16 * P, PACK16 * SW16C], F16, kind="ExternalInput")
    w1d = nc.dram_tensor("w1d", [NW1 * P, W1PACK * KD], FP8E3, kind="ExternalInput")
    w2d = nc.dram_tensor("w2d", [NW2 * P, KCE * 1024], FP8E3, kind="ExternalInput")
    # everything else (scalar/gpsimd rings)
    xt = nc.dram_tensor("xt", [P, 3, NTOK + 1], F32, kind="ExternalInput")
    x2b = nc.dram_tensor("x2b", [NTOK, D], F16, kind="ExternalInput")
    xft = nc.dram_tensor("xft", [P, KCS, B], F16, kind="ExternalInput")
    sw2 = nc.dram_tensor("sw2", [P, KCH, E], F16, kind="ExternalInput")
    ch1 = nc.dram_tensor("ch1", [P, KCE * CH1C], F16, kind="ExternalInput")
    ch2 = nc.dram_tensor("ch2", [P, 3, C], F16, kind="ExternalInput")
    oh = nc.dram_tensor("oh", [P, E], F32, kind="ExternalInput")
    c2d = nc.dram_tensor("c2d", [1, KD], F16, kind="ExternalInput")
    onesd = nc.dram_tensor("onesd", [P, 1], F16, kind="ExternalInput")
    if include_bias:
        b1d = nc.dram_tensor("b1d", [1, KD], F32, kind="ExternalInput")      # half-split
        b2d = nc.dram_tensor("b2d", [1, KD], F32, kind="ExternalInput")      # pre-scaled x128
        swb1d = nc.dram_tensor("swb1d", [1, SWC], F32, kind="ExternalInput")
        swb2d = nc.dram_tensor("swb2d", [1, E], F32, kind="ExternalInput")
        chb1d = nc.dram_tensor("chb1d", [1, CH1C], F32, kind="ExternalInput")
    outp = nc.dram_tensor("outp", [B, C], F32, kind="ExternalOutput")
    if DEBUG_DUMP:
        h1_dbg = nc.dram_tensor("h1_dbg", [P, SWC // 2], F16, kind="ExternalOutput")
        h_dbg = nc.dram_tensor("h_dbg", [P, KD // 2], F16, kind="ExternalOutput")
        er_dbg = nc.dram_tensor("er_dbg", [P, KD // 2], F16, kind="ExternalOutput")
        ws_dbg = nc.dram_tensor("ws_dbg", [P, KCE, B], F16, kind="ExternalOutput")
        z_dbg = nc.dram_tensor("z_dbg", [P, E], F32, kind="ExternalOutput")
        hb_dbg = nc.dram_tensor("hb_dbg", [1, B], F16, kind="ExternalOutput")

    with tile.TileContext(nc) as tc:
        with (
            tc.tile_pool(name="consts", bufs=1) as consts,
            tc.tile_pool(name="acts", bufs=1) as acts,
            tc.tile_pool(name="wpool", bufs=6) as wpool,
            tc.tile_pool(name="ps", bufs=6, space="PSUM") as ps,
            tc.tile_pool(name="dram", bufs=1, space="DRAM") as dram,
        ):
            # ---- constants / inputs on the scalar+gpsimd rings ----
            ident = consts.tile([P, P], F16)
            make_identity(nc, ident[:])
            # only xft (first sw matmuls) + xt (router) load up-front on the
            # scalar ring; everything else is emitted later so the early
            # scalar FIFO stays clear for the router lg round-trip.
            xft_sb = consts.tile([P, KCS, B], F16)
            nc.scalar.dma_start(xft_sb[:], xft[:])
            # xt shares its (large) SBUF slot with the later ch1: the router
            # is done with xt long before ch1 streams in.
            xt_sb = acts.tile([P, 3, NTOK + 1], F32, tag="xtch")
            nc.scalar.dma_start(xt_sb[:], xt[:])
            sw2_sb = consts.tile([P, KCH, E], F16)
            oh_sb = consts.tile([P, E], F32)
            ch2_sb = consts.tile([P, 3, C], F16)
            c2_sb = consts.tile([1, KD], F16)
            ones_sb = consts.tile([P, 1], F16)
            if include_bias:
                # half-split layouts: top partitions get block A columns,
                # bottom partitions block B (matching the paired psum halves)
                b1_sb = consts.tile([P, KD // 2], F32)
                nc.scalar.dma_start(b1_sb[0:B, :], b1d[0:1, 0:KD // 2].to_broadcast([B, KD // 2]))
                nc.scalar.dma_start(b1_sb[B:P, :], b1d[0:1, KD // 2:KD].to_broadcast([B, KD // 2]))
                b2_sb = consts.tile([P, KD // 2], F32)
                nc.scalar.dma_start(b2_sb[0:B, :], b2d[0:1, 0:KD // 2].to_broadcast([B, KD // 2]))
                nc.scalar.dma_start(b2_sb[B:P, :], b2d[0:1, KD // 2:KD].to_broadcast([B, KD // 2]))
                swb1_sb = consts.tile([P, SWC // 2], F32)
                nc.scalar.dma_start(swb1_sb[0:B, :], swb1d[0:1, 0:SWC // 2].to_broadcast([B, SWC // 2]))
                nc.scalar.dma_start(swb1_sb[B:P, :], swb1d[0:1, SWC // 2:SWC].to_broadcast([B, SWC // 2]))
                swb2_sb = consts.tile([P, E], F32)
                nc.scalar.dma_start(swb2_sb[:], swb2d[0:1, :].to_broadcast([P, E]))
                chb1_sb = consts.tile([B, CH1C], F32)
                nc.scalar.dma_start(chb1_sb[:], chb1d[0:1, :].to_broadcast([B, CH1C]))

            pwarm = ps.tile([P, B], F16, name="pwarm", tag="pt", bufs=2)
            nc.tensor.transpose(pwarm[:32, :32], ident[:32, :32], ident[:32, :32])

            identB = ident[B:P, B:P]  # I_64 on partitions 64..127

            # ---- sum-weights GEMM1: out [64, 1536] as 4 col-blocks of 384;
            # pairs (blk0->pms0 top, blk2->pms0 bottom), (blk1->pms1 top,
            # blk3->pms1 bottom). h1 layout: top partitions = cols 0:768,
            # bottom = cols 768:1536. ----
            pmsA = [ps.tile([P, 384], F32, name=f"pmsA{r}", tag="pm")
                    for r in range(2)]
            pmsB = [ps.tile([P, 384], F32, name=f"pmsB{r}", tag="pm")
                    for r in range(2)]

            wt8 = wt16 = None
            for c in range(KCS):
                if c % PACK8 == 0:
                    wt8 = wpool.tile([P, PACK8 * SW8C], FP8E3, name="wt8", tag="wt")
                    nc.sync.dma_start(wt8[:], swd8[ts(c // PACK8, P), :])
                if c % PACK16 == 0:
                    wt16 = wpool.tile([P, PACK16 * SW16C], F16, name="wt16", tag="wt")
                    nc.sync.dma_start(wt16[:], swd16[ts(c // PACK16, P), :])
                for r in range(2):
                    nc.tensor.matmul(
                        pmsA[r][0:B, :], xft_sb[:, c, :],
                        wt8[:, ds((c % PACK8) * SW8C + r * 384, 384)],
                        start=(c == 0), stop=(c == KCS - 1),
                        tile_position=(0, 0),
                    )
                    nc.tensor.matmul(
                        pmsB[r][B:P, :], xft_sb[:, c, :],
                        wt16[:, ds((c % PACK16) * SW16C + r * 384, 384)],
                        start=(c == 0), stop=(c == KCS - 1),
                        tile_position=(0, B),
                    )
                if c == 8:
                    # ---- router (fp32): logits = x @ emb_e; the DRAM
                    # round-trip rides the gpsimd ring (scalar FIFO is busy) ----
                    lg_flat = acts.tile([1, NTOK], F32)
                    for nt in range(4):
                        pr = ps.tile([B, 512], F32, name="pr", tag="pt", bufs=2)
                        for cc in range(3):
                            nc.tensor.matmul(
                                pr[:1, :], xt_sb[:, cc, NTOK : NTOK + 1],
                                xt_sb[:, cc, ts(nt, 512)],
                                start=(cc == 0), stop=(cc == 2),
                            )
                        nc.vector.tensor_copy(lg_flat[:, ts(nt, 512)], pr[:1, :])
                    lg_dram = dram.tile([1, NTOK], F32)
                    nc.gpsimd.dma_start(lg_dram[:], lg_flat[:])
                    lg_bn = acts.tile([B, N], F32)
                    nc.gpsimd.dma_start(
                        lg_bn[:], lg_dram[:].rearrange("x (b n) -> (x b) n", b=B))
                if c == 16:
                    # ---- top-8 per row + token gather (x2b is f16 x/128) ----
                    vals8 = acts.tile([B, 8], F32)
                    idx8 = acts.tile([B, 8], U32)
                    nc.vector.max(out=vals8[:], in_=lg_bn[:])
                    nc.vector.max_index(out=idx8[:], in_max=vals8[:], in_values=lg_bn[:])
                    base = acts.tile([B, 1], U32)
                    nc.gpsimd.iota(base[:], pattern=[[0, 1]], base=0, channel_multiplier=N)
                    off = acts.tile([B, 8], U32)
                    nc.vector.tensor_tensor(
                        out=off[:], in0=idx8[:], in1=base[:].to_broadcast([B, 8]), op=ADD)
                    sel = acts.tile([B, K, D], F16)
                    for k in range(K):
                        nc.gpsimd.indirect_dma_start(
                            out=sel[:, k, :], out_offset=None,
                            in_=x2b[:],
                            in_offset=bass.IndirectOffsetOnAxis(ap=off[:, k : k + 1], axis=0),
                        )
            # selT transposes AFTER the sw loop: emitting them mid-loop
            # parks the PE stream on the (slow) gather chain and stalls the
            # whole weight pipeline behind it.
            sel_flat = sel[:].rearrange("b k d -> b (k d)")
            selT = acts.tile([P, KCE, B], F16)
            for c in range(KCE):
                pt = ps.tile([P, B], F16, name="pt", tag="pt", bufs=2)
                nc.tensor.transpose(pt[:], sel_flat[:, ts(c, P)], ident[:B, :B])
                nc.vector.tensor_copy(selT[:, c, :], pt[:])

            nc.scalar.dma_start(sw2_sb[:], sw2[:])

            # ---- h1 = gelu(psum_sw); half-split layout [128, 768] ----
            h1 = acts.tile([P, SWC // 2], F16)
            for r in range(2):
                if include_bias:
                    nc.vector.tensor_add(
                        pmsA[r][0:B, :], pmsA[r][0:B, :], swb1_sb[0:B, ts(r, 384)])
                    nc.vector.tensor_add(
                        pmsB[r][B:P, :], pmsB[r][B:P, :], swb1_sb[B:P, ts(r, 384)])
                nc.scalar.activation(h1[0:B, ts(r, 384)], pmsA[r][0:B, :], GELU)
                nc.scalar.activation(h1[B:P, ts(r, 384)], pmsB[r][B:P, :], GELU)
            h1T = acts.tile([P, KCH, B], F16)
            for c in range(KCH):
                pt = ps.tile([P, B], F16, name="pt", tag="pt", bufs=2)
                if c < KCH // 2:
                    nc.tensor.transpose(pt[:], h1[0:B, ts(c, P)], ident[:B, :B])
                else:
                    nc.tensor.transpose(pt[:], h1[B:P, ts(c - KCH // 2, P)], identB)
                nc.vector.tensor_copy(h1T[:, c, :], pt[:])

            # ---- z partials + AllReduce (tiny, fp32) ----
            pz = ps.tile([B, 512], F32, name="pz", tag="pm")
            for c in range(KCH):
                nc.tensor.matmul(
                    pz[:E, :B], sw2_sb[:, c, :], h1T[:, c, :],
                    start=(c == 0), stop=(c == KCH - 1),
                )
            zT_sb = acts.tile([E, B], F32)
            nc.vector.tensor_copy(zT_sb[:], pz[:E, :B])
            zin = dram.tile([E, B], F32)
            zout = dram.tile([E, B], F32, addr_space="Shared")
            nc.gpsimd.dma_start(zin[:], zT_sb[:])
            nc.gpsimd.collective_compute(
                "AllReduce", ADD, replica_groups=[list(range(NCORES))],
                ins=[zin[:].opt()], outs=[zout[:].opt()],
            )

            # ---- expert GEMM1: h = gelu(selT.T @ w1_e), paired 3 rounds;
            # h layout [128, 1536]: top = cols 0:1536, bottom = 1536:3072 ----
            h = acts.tile([P, KD // 2], F16)
            pmeA = [ps.tile([P, 512], F32, name=f"pmeA{r}", tag="pm")
                    for r in range(3)]
            pmeB = [ps.tile([P, 512], F32, name=f"pmeB{r}", tag="pm")
                    for r in range(3)]
            for t in range(NW1):
                wt = wpool.tile([P, W1PACK * KD], FP8E3, name="wt", tag="wt")
                nc.sync.dma_start(wt[:], w1d[ts(t, P), :])
                if t == 1:
                    ch1_sb = acts.tile([P, KCE * CH1C], F16, tag="xtch")
                    nc.scalar.dma_start(ch1_sb[:], ch1[:])
                    nc.scalar.dma_start(ch2_sb[:], ch2[:])
                    nc.scalar.dma_start(c2_sb[:], c2d[:])
                    nc.scalar.dma_start(ones_sb[:], onesd[:])
                    nc.scalar.dma_start(oh_sb[:], oh[:])
                for k in range(W1PACK):
                    c = t * W1PACK + k
                    for r in range(3):
                        nc.tensor.matmul(
                            pmeA[r][0:B, :], selT[:, c, :],
                            wt[:, ds(k * KD + r * 512, 512)],
                            start=(c == 0), stop=(c == KCE - 1),
                            tile_position=(0, 0),
                        )
                        nc.tensor.matmul(
                            pmeB[r][B:P, :], selT[:, c, :],
                            wt[:, ds(k * KD + 1536 + r * 512, 512)],
                            start=(c == 0), stop=(c == KCE - 1),
                            tile_position=(0, B),
                        )
            last_gelu = None
            for r in range(3):
                if include_bias:
                    nc.vector.tensor_add(
                        pmeA[r][0:B, :], pmeA[r][0:B, :], b1_sb[0:B, ts(r, 512)])
                    nc.vector.tensor_add(
                        pmeB[r][B:P, :], pmeB[r][B:P, :], b1_sb[B:P, ts(r, 512)])
                nc.scalar.activation(h[0:B, ts(r, 512)], pmeA[r][0:B, :], GELU)
                last_gelu = nc.scalar.activation(
                    h[B:P, ts(r, 512)], pmeB[r][B:P, :], GELU)
            hT = acts.tile([P, KCE, B], F16)
            last_htc = None
            for c in range(KCE):
                pt = ps.tile([P, B], F16, name="pt", tag="pt", bufs=2)
                if c < KCE // 2:
                    nc.tensor.transpose(pt[:], h[0:B, ts(c, P)], ident[:B, :B])
                else:
                    nc.tensor.transpose(pt[:], h[B:P, ts(c - KCE // 2, P)], identB)
                last_htc = nc.vector.tensor_copy(hT[:, c, :], pt[:])

            # ---- hbar[b] = sum_i h[b, i] via ones-matmul over hT ----
            phb = ps.tile([1, B], F32, name="phb", tag="pt", bufs=2)
            for c in range(KCE):
                nc.tensor.matmul(
                    phb[:1, :], ones_sb[:, 0:1], hT[:, c, :],
                    start=(c == 0), stop=(c == KCE - 1),
                )
            hbar_sb = acts.tile([1, B], F16)
            hbar_cp = nc.vector.tensor_copy(hbar_sb[:], phb[:1, :])

            # ---- softmax over experts, duplicated on both partition halves
            # so the er scale covers [128, .] in one op. wes2 = w[:, e]/128 ----
            zb = acts.tile([P, E], F32)
            nc.gpsimd.dma_start(zb[0:B, :], zout[:].rearrange("e b -> b e"))
            nc.gpsimd.dma_start(zb[B:P, :], zout[:].rearrange("e b -> b e"))
            if include_bias:
                nc.vector.tensor_add(zb[:], zb[:], swb2_sb[:])
            mx = acts.tile([P, 1], F32)
            mx_i = nc.vector.reduce_max(mx[:], zb[:], axis=X_AX)
            tile.add_dep_helper(mx_i.ins, last_htc.ins, sync=False,
                                reason="softmax after hT copies on DVE")
            tile.add_dep_helper(mx_i.ins, hbar_cp.ins, sync=False,
                                reason="hbar cast must precede z-gated softmax "
                                       "on the DVE FIFO (corr MMs need it)")
            nmx = acts.tile([P, 1], F32)
            nc.vector.tensor_scalar_mul(nmx[:], mx[:], -1.0)
            exps = acts.tile([P, E], F32)
            exp_i = nc.scalar.activation(exps[:], zb[:], EXP, bias=nmx[:])
            tile.add_dep_helper(exp_i.ins, last_gelu.ins, sync=False,
                                reason="Exp after expert gelus on ACT")
            sm = acts.tile([P, 1], F32)
            nc.vector.reduce_sum(sm[:], exps[:], axis=X_AX)
            rs = acts.tile([P, 1], F32)
            nc.vector.reciprocal(rs[:], sm[:])
            wv = acts.tile([P, E], F32)
            nc.vector.tensor_scalar_mul(wv[:], exps[:], rs[:])
            t8 = acts.tile([P, E], F32)
            nc.vector.tensor_mul(out=t8[:], in0=wv[:], in1=oh_sb[:])
            wes2 = acts.tile([P, 1], F32)
            nc.vector.reduce_sum(wes2[:], t8[:], axis=X_AX)
            nc.vector.tensor_scalar_mul(wes2[:], wes2[:], 1.0 / SCALE)

            # ---- expert GEMM2, n-major paired: tile r holds blocks (r, r+3)
            # for all 24 k-chunks; er layout [128, 1536]: top = cols 0:1536,
            # bottom = 1536:3072. Rank-1 fp8 correction rows close each bank. ----
            cin = dram.tile([P, KD // 2], F16, name="cin")
            rsout = dram.tile([P // NCORES, KD // 2], F16, name="rsout")
            wsout = dram.tile([P, KD // 2], F16, name="wsout", addr_space="Shared")
            er_sb = acts.tile([P, KD // 2], F16)
            pme2A = [ps.tile([P, 512], F32, name=f"pme2A{r}", tag="pm")
                     for r in range(3)]
            pme2B = [ps.tile([P, 512], F32, name=f"pme2B{r}", tag="pm")
                     for r in range(3)]
            for r in range(NW2):
                wt = wpool.tile([P, KCE * 1024], FP8E3, name="wt2", tag="w2t", bufs=2)
                nc.sync.dma_start(wt[:], w2d[ts(r, P), :])
                lastA = lastB = None
                for k in range(KCE):
                    lastA = nc.tensor.matmul(
                        pme2A[r][0:B, :], hT[:, k, :],
                        wt[:, ds(k * 1024, 512)],
                        start=(k == 0), stop=False,
                        tile_position=(0, 0),
                    )
                    lastB = nc.tensor.matmul(
                        pme2B[r][B:P, :], hT[:, k, :],
                        wt[:, ds(k * 1024 + 512, 512)],
                        start=(k == 0), stop=False,
                        tile_position=(0, B),
                    )
                # the scheduler treats psum accumulates as commutative and
                # would hoist the (early-ready) correction MM before the
                # start=True MM, which wipes it -- pin it after the k-loop.
                corrA = nc.tensor.matmul(
                    pme2A[r][0:B, :], hbar_sb[0:1, :],
                    c2_sb[0:1, ds(r * 512, 512)],
                    start=False, stop=True,
                    tile_position=(0, 0),
                )
                tile.add_dep_helper(corrA.ins, lastA.ins, sync=False,
                                    reason="corr row after bank A k-loop")
                corrB = nc.tensor.matmul(
                    pme2B[r][B:P, :], hbar_sb[0:1, :],
                    c2_sb[0:1, ds(1536 + r * 512, 512)],
                    start=False, stop=True,
                    tile_position=(0, B),
                )
                tile.add_dep_helper(corrB.ins, lastB.ins, sync=False,
                                    reason="corr row after bank B k-loop")
                if include_bias:
                    nc.vector.tensor_add(
                        pme2A[r][0:B, :], pme2A[r][0:B, :], b2_sb[0:B, ts(r, 512)])
                    nc.vector.tensor_add(
                        pme2B[r][B:P, :], pme2B[r][B:P, :], b2_sb[B:P, ts(r, 512)])
                nc.vector.tensor_scalar_mul(
                    er_sb[0:B, ts(r, 512)], pme2A[r][0:B, :], wes2[0:B, :])
                nc.vector.tensor_scalar_mul(
                    er_sb[B:P, ts(r, 512)], pme2B[r][B:P, :], wes2[B:P, :])
                nc.scalar.dma_start(cin[:, ts(r, 512)], er_sb[:, ts(r, 512)])
            pmh = ps.tile([B, 512], F32, name="pmh", tag="pm")
            nc.gpsimd.collective_compute(
                "ReduceScatter", ADD, replica_groups=[list(range(NCORES))],
                ins=[cin[:].opt()], outs=[rsout[:].opt()],
            )
            nc.gpsimd.collective_compute(
                "AllGather", mybir.AluOpType.bypass,
                replica_groups=[list(range(NCORES))],
                ins=[rsout[:].opt()], outs=[wsout[:].opt()],
            )

            # ---- head GEMM1 (column shard): wsT via 2 XBAR DMA transposes
            # (top half = ws cols 0:1536 -> chunks 0:12, bottom -> 12:24) ----
            wsT = acts.tile([P, KCE, B], F16)
            nc.scalar.dma_start(wsT[:, 0 : KCE // 2, :], wsout[0:B, :], transpose=True)
            nc.scalar.dma_start(wsT[:, KCE // 2 : KCE, :], wsout[B:P, :], transpose=True)
            for c in range(KCE):
                nc.tensor.matmul(
                    pmh[:, :CH1C], wsT[:, c, :],
                    ch1_sb[:, ds(c * CH1C, CH1C)],
                    start=(c == 0), stop=(c == KCE - 1),
                )
            hh = acts.tile([B, CH1C], F16)
            if include_bias:
                nc.vector.tensor_add(pmh[:, :CH1C], pmh[:, :CH1C], chb1_sb[:])
            nc.scalar.activation(hh[:], pmh[:, :CH1C], GELU)

            # ---- head GEMM2 (contraction shard): out_part = hh @ ch2_e ----
            hhT = acts.tile([P, 3, B], F16)
            nc.scalar.dma_start(hhT[:], hh[:], transpose=True)
            outsb = acts.tile([B, C], F32)
            for nn in range(2):
                pmo = ps.tile([B, 512], F32, name="pmo", tag="pm")
                for c in range(3):
                    nc.tensor.matmul(
                        pmo[:, :500], hhT[:, c, :], ch2_sb[:, c, ds(nn * 500, 500)],
                        start=(c == 0), stop=(c == 2),
                    )
                nc.vector.tensor_copy(outsb[:, ds(nn * 500, 500)], pmo[:, :500])
            nc.scalar.dma_start(outp[:], outsb[:])

            if DEBUG_DUMP:
                nc.scalar.dma_start(h1_dbg[:], h1[:])
                nc.scalar.dma_start(h_dbg[:], h[:])
                nc.scalar.dma_start(er_dbg[:], er_sb[:])
                nc.scalar.dma_start(ws_dbg[:], wsT[:])
                nc.scalar.dma_start(z_dbg[:], zb[:])
                nc.scalar.dma_start(hb_dbg[:], hbar_sb[:])

    nc.finalize()
    return nc


_NC_CACHE: dict = {}


def _get_nc(include_bias: bool) -> bass.Bass:
    if include_bias not in _NC_CACHE:
        _NC_CACHE[include_bias] = _build(include_bias)
    return _NC_CACHE[include_bias]


def _q8(w: np.ndarray) -> np.ndarray:
    return np.clip(w * SCALE, -M8, M8).astype(f8e3)


def _pack_inputs(inputs: dict, include_bias: bool) -> list[dict]:
    f32 = np.float32
    x = np.ascontiguousarray(np.asarray(inputs["x"], dtype=f32))      # (64,32,384)
    expert_emb = np.asarray(inputs["expert_emb"], dtype=f32)          # (8,384)
    w1 = np.asarray(inputs["w1"])                                     # (8,3072,3072)
    w2 = np.asarray(inputs["w2"])
    sw_w1 = np.asarray(inputs["sw_w1"])                               # (12288,12288)
    sw_w2 = np.asarray(inputs["sw_w2"])                               # (12288,8)
    ch_w1 = np.asarray(inputs["ch_w1"])                               # (3072,3072)
    ch_w2 = np.asarray(inputs["ch_w2"])                               # (3072,1000)

    x2 = x.reshape(NTOK, D)
    xt_base = x2.T.reshape(3, P, NTOK).transpose(1, 0, 2)             # (128,3,2048)
    x2b = (x2 / SCALE).astype(f16)                                    # gather source
    xf = x.reshape(B, ND)
    xft_p = np.ascontiguousarray(
        (xf.T / SCALE).reshape(KCS, P, B).transpose(1, 0, 2)).astype(f16)

    ch1_full = ch_w1.reshape(KD, E, CH1C)                             # col shards
    ch2_full = ch_w2.reshape(E, CH1C, C)                              # row shards
    ones_p = np.ones((P, 1), dtype=f16)

    in_maps = []
    for e in range(NCORES):
        emb_p = expert_emb[e].reshape(3, P).T                          # (128,3)
        xt_p = np.ascontiguousarray(
            np.concatenate([xt_base, emb_p[:, :, None]], axis=2), dtype=f32)
        # importance-sorted column split: fp8 takes the 768 columns whose
        # sw_w2 rows have the smallest norm (lowest impact on z).
        sw2_e_f = np.asarray(sw_w2[e * SWC:(e + 1) * SWC, :], f32)
        perm = np.argsort(np.linalg.norm(sw2_e_f, axis=1), kind="stable")
        sw1_e = np.asarray(sw_w1[:, e * SWC:(e + 1) * SWC], f32)[:, perm] * SCALE
        swd8_p = np.ascontiguousarray(
            np.clip(sw1_e[:, :SW8C], -M8, M8).astype(f8e3)
            .reshape(NSW8, PACK8, P, SW8C).transpose(0, 2, 1, 3)
            .reshape(NSW8 * P, PACK8 * SW8C))
        swd16_p = np.ascontiguousarray(
            sw1_e[:, SW8C:].astype(f16)
            .reshape(NSW16, PACK16, P, SW16C).transpose(0, 2, 1, 3)
            .reshape(NSW16 * P, PACK16 * SW16C))
        w1d_p = np.ascontiguousarray(
            _q8(np.asarray(w1[e], f32)).reshape(NW1, W1PACK, P, KD)
            .transpose(0, 2, 1, 3).reshape(NW1 * P, W1PACK * KD))
        w2_e = np.asarray(w2[e], f32)
        w2q = _q8(w2_e)                                                # (3072,3072) fp8
        delta2 = w2q.astype(f32) / SCALE - w2_e
        c2_p = (-(SCALE / KD) * delta2.sum(axis=0)).astype(f16).reshape(1, KD)
        w2k = w2q.reshape(KCE, P, KD)
        w2d_p = np.ascontiguousarray(np.stack([
            np.concatenate(
                [w2k[:, :, 512 * r:512 * r + 512],
                 w2k[:, :, 1536 + 512 * r:1536 + 512 * r + 512]], axis=2)
            .transpose(1, 0, 2).reshape(P, KCE * 1024)
            for r in range(NW2)]).reshape(NW2 * P, KCE * 1024))
        sw2_perm = sw2_e_f[perm]                                       # (1536,8)
        sw2_p = np.ascontiguousarray(
            sw2_perm.reshape(KCH, P, E).transpose(1, 0, 2)).astype(f16)  # (128,12,8)
        ch1_p = np.ascontiguousarray(
            ch1_full[:, e, :].reshape(KCE, P, CH1C).transpose(1, 0, 2)
            .reshape(P, KCE * CH1C)).astype(f16)                       # (128,24*384)
        ch2_p = np.ascontiguousarray(
            ch2_full[e].reshape(3, P, C).transpose(1, 0, 2)).astype(f16)
        oh_p = np.zeros((P, E), dtype=f32)
        oh_p[:, e] = 1.0
        m = {
            "xt": xt_p, "x2b": x2b, "xft": xft_p,
            "swd8": swd8_p, "swd16": swd16_p, "w1d": w1d_p, "w2d": w2d_p,
            "sw2": sw2_p, "ch1": ch1_p, "ch2": ch2_p, "oh": oh_p,
            "c2d": c2_p, "onesd": ones_p,
        }
        if include_bias:
            m["b1d"] = np.asarray(inputs["b1"][e], np.float32).reshape(1, KD)
            m["b2d"] = np.asarray(inputs["b2"][e], np.float32).reshape(1, KD) * SCALE
            m["swb1d"] = np.asarray(
                inputs["sw_b1"], np.float32).reshape(1, ND)[:, e * SWC:(e + 1) * SWC][:, perm]
            m["swb2d"] = np.asarray(inputs["sw_b2"], np.float32).reshape(1, E)
            m["chb1d"] = np.asarray(
                inputs["ch_b1"], np.float32).reshape(1, KD)[:, e * CH1C:(e + 1) * CH1C]
        in_maps.append(m)
    return in_maps


def _need_bias(inputs) -> bool:
    return any(
        float(np.abs(np.asarray(inputs[k])).max()) != 0.0
        for k in ("b1", "b2", "sw_b1", "sw_b2", "ch_b1")
    )


def run(inputs: dict, **run_kwargs):
    """Run on the 8 cores; returns (full_output, BassKernelResults)."""
    include_bias = _need_bias(inputs)
    nc = _get_nc(include_bias)
    in_maps = _pack_inputs(inputs, include_bias)
    res = run_bass_kernel_spmd(nc, in_maps, core_ids=list(range(NCORES)), **run_kwargs)
    out = np.zeros((B, C), dtype=np.float64)
    for e in range(NCORES):
        out += res.results[e]["outp"].astype(np.float64)
    out += np.asarray(inputs["ch_b2"], np.float64)
    return out.astype(np.float32), res


def kernel(**inputs) -> np.ndarray:
    out, _ = run(inputs)
    return out
